# revision 8
# baseline (speedup 1.0000x reference)
"""Trainium2 Bass kernel for nn_DilatedOCA (dilated overlapping cross-attention).

Math (per reference):
  xn = x / sqrt(var(x, ch) + 1e-5) * ln_w           (bias-free LN over channels)
  qkv = w_qkv @ xn (1x1 conv); q/k/v split
  q: [heads, N=4096, 16] from channels
  k,v: torch-unfold(12x12 win, stride 8, pad 2) + a reshape that scrambles
       (channel, window-pos, window-idx) into [heads, M=9216, 16] where the
       "16" dim is the LOW 4 BITS OF THE WINDOW INDEX (faithful to source).
  attn = softmax(q k^T / 4) over all M; out = attn @ v; final 1x1 conv w_out.

Index algebra (head h, d = 8a+s with a=d//8, s=d%8):
  K^T[d, m] = k_pad[16h+ci, 16lq+8a+kh, 8s+kw]   m=(ci,kh,kw,lq)
  V[m, d]   = v_pad[16h+ci, 16lq+8a+kh, 8s+kw]
  Q^T[d, n] = q[16h+d, n]
Softmax/PV are invariant to any fixed permutation of m, so we use our own
enumeration  m' = ((((khH*3+khL)*4+lq)*6+kw1)*2+kw0)*16+ci  (kh=3khH+khL,
kw=2kw1+kw0), which makes the K/V gather DMAs contiguous 192-element runs.

Sharding: 8 cores = 4 heads x 2 query-halves (n in [0,2048) / [2048,4096)).

Main loop (per 512-query chunk nck, 24 slots of 3 key-tiles):
  QK: 3 matmuls row-packed into PE 32-row groups (K^T replicated at
      partitions 0/32/64) -> one [128,1536] f32 PSUM strip.
  exp: split across engines. ACT slots: activation(Exp, bias=ln kappa).
      DVE slots: Schraudolph fp16 bit-trick - one tensor_scalar
      round(A*s+B) -> int16, bitcast fp16.  Pool upgrades some DVE slots to
      a phase-averaged pair (bits+512 = value*sqrt2, opposite sawtooth
      phase) contracted against vt2 = vt/sqrt2.
  PV: col-packed matmuls into 4 PSUM partition strips (T mod 4), 17th row
      of V = ones accumulates the softmax denominator.
  Final per nck: strip-sum via sel-matmul, reciprocal, w_out conv, and a
      ones-outer-product matmul to broadcast 1/den across partitions.
"""

import sys

for _p in ("/opt/trn_rl_repo", "/root/.axon_site/_ro/pypackages"):
    if _p not in sys.path:
        sys.path.insert(0, _p)

import numpy as np

import concourse.bass as bass
import concourse.mybir as mybir
import concourse.tile as tile
from concourse import bacc
from concourse.bass_utils import run_bass_kernel_spmd

F32 = mybir.dt.float32
PF16 = mybir.dt.float16  # fp16: same PE rate as bf16, 8x mantissa
I16 = mybir.dt.int16
AF = mybir.ActivationFunctionType
ALU = mybir.AluOpType

HEADS, DH = 4, 16
NPIX, NHALF = 4096, 2048
PADW = 68          # padded image height/width
PFREE = PADW * 16  # padT3 free size: col*16 + ci = 1088
M = 9216           # keys per head
NT = 72            # m' tiles of 128
EPS = 1e-5

# exp approximation constants (fp16 Schraudolph, round-to-nearest)
EXPA = 1477.3197218702985          # 1024*log2(e)
EXPB = 15285.0                     # averaged slots (sigma=75)
EXPB_NA = 16309.169871931477       # plain-DVE slots, matches pair mean
ACT_BIAS = 0.6822434815795542      # ln(kappa_pair)
INV_SQRT2 = 0.7071067811865476

N_ACT = 50   # of 96 slots on ACT (exact exp)
N_AVG = 27   # of the DVE slots, phase-averaged via Pool

_CACHE = {}


def _slot_schedule():
    """96 slots -> 'A' (ACT), 'D' (plain DVE), 'V' (averaged DVE)."""
    eng = []
    acc = 0
    for _ in range(96):
        acc += N_ACT
        if acc >= 96:
            acc -= 96
            eng.append("A")
        else:
            eng.append("D")
    n_dve = eng.count("D")
    acc2 = 0
    for i in range(96):
        if eng[i] == "D" and n_dve:
            acc2 += N_AVG
            if acc2 >= n_dve:
                acc2 -= n_dve
                eng[i] = "V"
    return eng


def _build(stage="full", reps=1):
    nc = bacc.Bacc(trn_type="TRN2")

    x_d = nc.dram_tensor("x", [64, NPIX], F32, kind="ExternalInput")
    xq_d = nc.dram_tensor("xq", [64, NHALF], F32, kind="ExternalInput")
    wkvT_d = nc.dram_tensor("wkvT", [64, 32], F32, kind="ExternalInput")
    wqT_d = nc.dram_tensor("wqT", [64, 16], F32, kind="ExternalInput")
    woutT_d = nc.dram_tensor("woutT", [16, 64], F32, kind="ExternalInput")
    ones1_d = nc.dram_tensor("ones1", [1, 64], F32, kind="ExternalInput")
    id128_d = nc.dram_tensor("id128", [128, 128], F32, kind="ExternalInput")
    id17_d = nc.dram_tensor("id17", [17, 17], F32, kind="ExternalInput")
    onesM_d = nc.dram_tensor("onesM", [1, M], F32, kind="ExternalInput")
    sel_d = nc.dram_tensor("sel", [128, 17], F32, kind="ExternalInput")
    y_d = nc.dram_tensor("y", [64, NHALF], F32, kind="ExternalOutput")
    ktmp_d = nc.dram_tensor("ktmp", [NPIX, 16], F32)
    vtmp_d = nc.dram_tensor("vtmp", [NPIX, 16], F32)

    sched = _slot_schedule()

    with tile.TileContext(nc) as tc:
        with tc.tile_pool(name="sb", bufs=1) as sb:
            # persistent sbuf tensors
            xsb = sb.tile([64, NPIX], F32)
            xqsb = sb.tile([64, NHALF], F32)
            padk = sb.tile([PADW, PFREE], F32)
            padv = sb.tile([PADW, PFREE], F32)
            gk = sb.tile([16, M], F32)
            gk4 = sb.tile([128, M], PF16)
            gv = sb.tile([17, M], F32)
            vt_all = sb.tile([128, 17 * NT], PF16)
            vt2 = sb.tile([128, 17 * NT], PF16)
            q4 = sb.tile([128, NHALF], PF16)
            stgkv = sb.tile([128, 1024], F32)
            stats = sb.tile([128, 96], F32)   # s1 cols 0:48, s2 cols 48:96
            rstdT = sb.tile([128, 48], F32)   # col t: chunk t (32 x, 16 xq)
            ysb = sb.tile([64, NHALF], F32)
            wkvT = sb.tile([64, 32], F32)
            wqT = sb.tile([64, 16], F32)
            woutT = sb.tile([16, 64], F32)
            ones1 = sb.tile([1, 64], F32)
            id128 = sb.tile([128, 128], F32)
            id17 = sb.tile([17, 17], F32)
            sel = sb.tile([128, 17], F32)
            abias = sb.tile([128, 1], F32)
            nc.gpsimd.memset(abias[:, :], ACT_BIAS)

            for _rep in range(reps):
                for dst, src in (
                    (xsb, x_d), (xqsb, xq_d), (wkvT, wkvT_d), (wqT, wqT_d),
                    (woutT, woutT_d), (ones1, ones1_d), (id128, id128_d),
                    (id17, id17_d), (sel, sel_d),
                ):
                    nc.sync.dma_start(out=dst[:, :], in_=src[:, :])

                # border zeros for padded images; ones row for softmax denom
                nc.gpsimd.memset(padk[:, :], 0.0)
                nc.gpsimd.memset(padv[:, :], 0.0)
                nc.sync.dma_start(out=gv[16:17, :], in_=onesM_d[:, :])

                with tc.tile_pool(name="sm", bufs=3) as sm, \
                     tc.tile_pool(name="pre", bufs=3, space="PSUM") as pre:

                    # ---- LN stats in transposed (pixel-partition) orient --
                    def chunk_src(t):
                        if t < 32:
                            return xsb[:, 128 * t:128 * (t + 1)]
                        return xqsb[:, 128 * (t - 32):128 * (t - 31)]

                    for t in range(48):
                        trp = pre.tile([128, 64], F32, tag="pre")
                        nc.tensor.transpose(trp[:, :], chunk_src(t),
                                            id128[0:64, 0:64])
                        xT = sm.tile([128, 64], F32, tag="xT")
                        nc.vector.tensor_copy(xT[:, :], trp[:, :])
                        nc.vector.reduce_sum(stats[:, t:t + 1], xT[:, :],
                                             axis=mybir.AxisListType.X)
                        scr = sm.tile([128, 64], F32, tag="scr")
                        nc.vector.tensor_mul(scr[:, :], xT[:, :], xT[:, :])
                        nc.vector.reduce_sum(stats[:, 48 + t:49 + t],
                                             scr[:, :],
                                             axis=mybir.AxisListType.X)

                    # rstd = 1/sqrt(s2/64 - (s1/64)^2 + eps)   [128, 48]
                    mean = sm.tile([128, 48], F32, tag="mean")
                    nc.vector.tensor_scalar_mul(mean[:, :], stats[:, 0:48],
                                                1.0 / 64)
                    nc.vector.tensor_mul(mean[:, :], mean[:, :], mean[:, :])
                    varr = sm.tile([128, 48], F32, tag="varr")
                    nc.vector.tensor_scalar_mul(varr[:, :], stats[:, 48:96],
                                                1.0 / 64)
                    nc.vector.tensor_sub(varr[:, :], varr[:, :], mean[:, :])
                    nc.vector.tensor_scalar_add(varr[:, :], varr[:, :], EPS)
                    nc.scalar.activation(rstdT[:, :], varr[:, :], AF.Sqrt)
                    nc.vector.reciprocal(rstdT[:, :], rstdT[:, :])

                    # ---- k,v 1x1 conv on RAW x; rstd folded via psum scale -
                    for t in range(32):
                        kv = pre.tile([128, 32], F32, tag="pre")
                        nc.tensor.matmul(kv[:, :],
                                         xsb[:, 128 * t:128 * (t + 1)],
                                         wkvT[:, :], start=True, stop=True)
                        nc.vector.tensor_scalar_mul(
                            stgkv[:, 32 * t:32 * (t + 1)], kv[:, :],
                            rstdT[:, t:t + 1])

                    # stgkv[p, 32t + c0 + ci] = (k|v)[ci, pixel=128t+p]
                    # -> (k|v)tmp[pixel, ci]  (DRAM, pixel-major)
                    for tmp_d, c0 in ((ktmp_d, 0), (vtmp_d, 16)):
                        src_ap = bass.AP(tensor=stgkv.tensor, offset=c0,
                                         ap=[[1024, 128], [32, 32], [1, 16]])
                        dst_ap = bass.AP(tensor=tmp_d, offset=0,
                                         ap=[[16, 128], [2048, 32], [1, 16]])
                        nc.sync.dma_start(out=dst_ap, in_=src_ap)
                    # -> pad[row, (col+2)*16 + ci] interior (+2 offsets)
                    for tmp_d, pad_t in ((ktmp_d, padk), (vtmp_d, padv)):
                        src_ap = bass.AP(tensor=tmp_d, offset=0,
                                         ap=[[1024, 64], [1, 1024]])
                        dst_ap = bass.AP(tensor=pad_t.tensor,
                                         offset=2 * PFREE + 2 * 16,
                                         ap=[[PFREE, 64], [1, 1024]])
                        nc.sync.dma_start(out=dst_ap, in_=src_ap)

                    # ---- q conv (head slice, 0.25 prefolded), pixel-part --
                    for t in range(16):
                        qp = pre.tile([128, 16], F32, tag="pre")
                        nc.tensor.matmul(qp[:, :],
                                         xqsb[:, 128 * t:128 * (t + 1)],
                                         wqT[:, :], start=True, stop=True)
                        qTc = sm.tile([128, 16], F32, tag="qTc")
                        nc.vector.tensor_scalar_mul(qTc[:, :], qp[:, :],
                                                    rstdT[:, 32 + t:33 + t])
                        qp2 = pre.tile([16, 128], F32, tag="pre")
                        nc.tensor.transpose(qp2[:, :], qTc[:, :], id128[:, :])
                        nc.vector.tensor_copy(q4[0:16,
                                                 128 * t:128 * (t + 1)],
                                              qp2[:, :])

                    # ---- gathers: pad -> G  (48 DMAs each) -----------------
                    # G[8a+s, m'] = pad[16lq+3khH+khL+8a, (8s+2kw1+kw0)*16+ci]
                    for pad_t, g_t in ((padk, gk), (padv, gv)):
                        for khH in range(4):
                            for khL in range(3):
                                for lq in range(4):
                                    row0 = 16 * lq + 3 * khH + khL
                                    src_ap = bass.AP(
                                        tensor=pad_t.tensor,
                                        offset=row0 * PFREE,
                                        ap=[[8 * PFREE, 2], [128, 8],
                                            [1, 192]])
                                    dst_ap = bass.AP(
                                        tensor=g_t.tensor,
                                        offset=(2304 * khH + 768 * khL
                                                + 192 * lq),
                                        ap=[[M, 16], [1, 192]])
                                    nc.sync.dma_start(out=dst_ap, in_=src_ap)

                    # fp16 cast of K^T; replicate to PE row groups 1,2
                    nc.vector.tensor_copy(gk4[0:16, :], gk[:, :])
                    nc.sync.dma_start(out=gk4[32:48, :], in_=gk4[0:16, :])
                    nc.scalar.dma_start(out=gk4[64:80, :], in_=gk4[0:16, :])
                    # replicate q to row groups 1,2
                    nc.sync.dma_start(out=q4[32:48, :], in_=q4[0:16, :])
                    nc.scalar.dma_start(out=q4[64:80, :], in_=q4[0:16, :])

                    # ---- V^T tiles: [17, 128] -> [128, 17] via PE ----------
                    for T in range(NT):
                        vtp = pre.tile([128, 17], F32, tag="pre")
                        nc.tensor.transpose(vtp[:, :],
                                            gv[:, 128 * T:128 * (T + 1)],
                                            id17[:, :])
                        nc.vector.tensor_copy(vt_all[:, 17 * T:17 * (T + 1)],
                                              vtp[:, :])
                    nc.vector.tensor_scalar_mul(vt2[:, :], vt_all[:, :],
                                                INV_SQRT2)

                _run_main(nc, tc, sb, sched, gk4, vt_all, vt2, q4, woutT,
                          ones1, sel, abias, ysb, y_d)

    nc.compile()
    return nc


def _run_main(nc, tc, sb, sched, gk4, vt_all, vt2, q4, woutT, ones1, sel,
              abias, ysb, y_d):
    with tc.tile_pool(name="stp", bufs=2, space="PSUM") as stp, \
         tc.tile_pool(name="pvp", bufs=1, space="PSUM") as pvp, \
         tc.tile_pool(name="fsp", bufs=1, space="PSUM") as fsp, \
         tc.tile_pool(name="ptp", bufs=3) as ptp, \
         tc.tile_pool(name="pt2p", bufs=2) as pt2p, \
         tc.tile_pool(name="fsb", bufs=2) as fsb:

        for nck in range(4):
            ncs = slice(512 * nck, 512 * (nck + 1))
            pv = pvp.tile([128, 512], F32, tag="pv")

            # per-strip first/last bookkeeping (start/stop flags)
            strip_first = {g: g for g in range(4)}          # T of first MM
            strip_last = {g: 68 + g for g in range(4)}      # T of last MM

            for j in range(24):
                slot = sched[24 * nck + j]
                st = stp.tile([128, 1536], F32, tag="st")
                for u in range(3):
                    T = 3 * j + u
                    nc.tensor.matmul(
                        st[:, 512 * u:512 * (u + 1)],
                        gk4[32 * u:32 * u + 16, 128 * T:128 * (T + 1)],
                        q4[32 * u:32 * u + 16, ncs],
                        start=True, stop=True)

                pt = ptp.tile([128, 1536], PF16, tag="pt")
                pt2 = None
                if slot == "A":
                    nc.scalar.activation(pt[:, :], st[:, :], AF.Exp,
                                         bias=abias[:, :])
                else:
                    bconst = EXPB if slot == "V" else EXPB_NA
                    nc.vector.tensor_scalar(
                        pt[:, :].bitcast(I16), st[:, :], EXPA, bconst,
                        ALU.mult, ALU.add)
                    if slot == "V":
                        pt2 = pt2p.tile([128, 1536], PF16, tag="pt2")
                        nc.gpsimd.tensor_scalar_add(
                            pt2[:, :].bitcast(I16), pt[:, :].bitcast(I16),
                            512)

                for u in range(3):
                    T = 3 * j + u
                    g = T % 4
                    first = (T == strip_first[g])
                    last = (T == strip_last[g])
                    nc.tensor.matmul(
                        pv[32 * g:32 * g + 17, :],
                        vt_all[:, 17 * T:17 * (T + 1)],
                        pt[:, 512 * u:512 * (u + 1)],
                        start=first, stop=(last and pt2 is None),
                        tile_position=(0, 32 * g))
                    if pt2 is not None:
                        nc.tensor.matmul(
                            pv[32 * g:32 * g + 17, :],
                            vt2[:, 17 * T:17 * (T + 1)],
                            pt2[:, 512 * u:512 * (u + 1)],
                            start=False, stop=last,
                            tile_position=(0, 32 * g))

            # ---- per-nck normalize + final 1x1 conv --------------------
            pvsb = fsb.tile([128, 512], F32, tag="pvsb")
            nc.scalar.add(pvsb[:, :], pv[:, :], 0.0)
            nd = fsp.tile([17, 512], F32, tag="fs")
            nc.tensor.matmul(nd[:, :], sel[:, :], pvsb[:, :],
                             start=True, stop=True)
            ndsb = fsb.tile([17, 512], F32, tag="ndsb")
            nc.scalar.add(ndsb[:, :], nd[:, :], 0.0)
            densb = fsb.tile([1, 512], F32, tag="densb")
            nc.sync.dma_start(out=densb[:, :], in_=ndsb[16:17, :])
            rden = fsb.tile([1, 512], F32, tag="rden")
            nc.vector.reciprocal(rden[:, :], densb[:, :])
            bp = fsp.tile([64, 512], F32, tag="fs")
            nc.tensor.matmul(bp[:, :], ones1[:, :], rden[:, :],
                             start=True, stop=True)
            bpsb = fsb.tile([64, 512], F32, tag="bpsb")
            nc.scalar.add(bpsb[:, :], bp[:, :], 0.0)
            yp = fsp.tile([64, 512], F32, tag="fs")
            nc.tensor.matmul(yp[:, :], woutT[:, :], ndsb[0:16, :],
                             start=True, stop=True)
            nc.vector.tensor_mul(ysb[:, ncs], yp[:, :], bpsb[:, :])

        nc.sync.dma_start(out=y_d[:, :], in_=ysb[:, :])


def _make_in_maps(x, w_qkv, w_out, ln_w):
    x2d = np.ascontiguousarray(x.reshape(64, NPIX))
    ones1 = np.ones((1, 64), np.float32)
    id128 = np.eye(128, dtype=np.float32)
    id17 = np.eye(17, dtype=np.float32)
    onesM = np.ones((1, M), np.float32)
    sel = np.zeros((128, 17), np.float32)
    for g in range(4):
        for jj in range(17):
            sel[32 * g + jj, jj] = 1.0

    in_maps = []
    for c in range(8):
        h, half = c % 4, c // 4
        wq = w_qkv[16 * h:16 * h + 16, :]
        wk = w_qkv[64 + 16 * h:64 + 16 * h + 16, :]
        wv = w_qkv[128 + 16 * h:128 + 16 * h + 16, :]
        lw = ln_w[None, :]
        in_maps.append({
            "x": x2d,
            "xq": np.ascontiguousarray(
                x2d[:, NHALF * half:NHALF * (half + 1)]),
            "wkvT": np.ascontiguousarray(
                (np.concatenate([wk, wv], 0) * lw).T.astype(np.float32)),
            "wqT": np.ascontiguousarray((0.25 * wq * lw).T.astype(np.float32)),
            "woutT": np.ascontiguousarray(
                w_out[:, 16 * h:16 * h + 16].T.astype(np.float32)),
            "ones1": ones1,
            "id128": id128,
            "id17": id17,
            "onesM": onesM,
            "sel": sel,
        })
    return in_maps


def _get_nc():
    if "nc" not in _CACHE:
        _CACHE["nc"] = _build()
    return _CACHE["nc"]


def kernel(x, w_qkv, w_out, ln_w, _want_trace=False):
    x = np.asarray(x, np.float32)
    w_qkv = np.asarray(w_qkv, np.float32)
    w_out = np.asarray(w_out, np.float32)
    ln_w = np.asarray(ln_w, np.float32)

    in_maps = _make_in_maps(x, w_qkv, w_out, ln_w)
    nc = _get_nc()
    res = run_bass_kernel_spmd(nc, in_maps, list(range(8)), trace=_want_trace)
    if _want_trace:
        _CACHE["last_result"] = res

    y = np.empty((64, NPIX), np.float32)
    for half in range(2):
        acc = np.zeros((64, NHALF), np.float32)
        for h in range(4):
            acc += res.results[4 * half + h]["y"]
        y[:, NHALF * half:NHALF * (half + 1)] = acc
    return y.reshape(1, 64, 64, 64)
